# revision 25
# baseline (speedup 1.0000x reference)
"""Gabor layer Trainium2 kernel — packed-tile formulation.

Per gabor g and pixel (x,y): amp[g,c] * exp(E) * cos(S + phase[g,c]) with E
quadratic and S affine in pixel coords. Using cos(S+p) = cos(p)cos(S) -
sin(p)sin(S), the channel sum over g is a matmul over the plane pair
(cos(S)*gauss, sin(S)*gauss) with contraction over gabors.

Key observation: with tile-centered features on a uniform grid, the matmul
rhs (tile-local monomials for E; row/col one-hots for S) is IDENTICAL for
every 16x32 tile — all per-tile variation lives in the stationary weight
tables. So rows of one 128-partition work unit can belong to DIFFERENT
tiles: each partition row is a (gabor, tile) pair. Gabors are culled per
tile (exact per-pixel E max, thr=1e-3), tiles are bin-packed ~4 per
128-row "pack", and every engine pass (matmul / Exp / Sin / products /
reduce) is amortized over all tiles in the pack.

E matmul runs in fp16 hi/lo (2 accumulated passes) instead of fp32 (2
half-speed passes): the feature basis f=(2c-31)/1024 etc. is chosen
fp16-EXACT, weights are scaled per row by a power of two into fp16 range
(quadratic coefficients reach 5e7 for thin gabors), and the inverse scale
rides the Exp activation's per-partition scale operand.

Device pipeline per pack (N=512 px):
    PE : E = (WEhi+WElo)^T @ feat6   (K=6 fp16 x2, row-scaled)
         S = WS^T @ onehot           (K=48 fp16: S[px] = A[g,row]+B[g,col])
    ACT: gauss = Exp(E * invscale)   (fp16)      [Exp table]
    DVE: w = add_range_wrap(S) in [-pi,pi)  (fp16; w == S mod 2pi)
    ACT: t2 = Sin(w*0.5)  (sin^2(w/2) == sin^2(S/2));  ss = Sin(w)
    DVE: u = t2*t2;  p1 = (u-0.5)*gauss = -cos(S)/2*gauss;  p2 = gauss*ss
    PE : po[60,512] += AB^T @ [p1;p2]  (fp16 zero-col-padded AB columns
         accumulate a 4-pack group into ONE psum bank)
    DVE: ob = copy(po);  1 DMA out per 4-pack group.
Final clamp + tile unscramble on host.

Sharding: each core owns a 64-row strip; the tile->pack map is shared
across cores (per-tile row budget = max gabor count over cores) so the
single SPMD program's baked addresses are valid on every core.
"""

import os
import sys

import numpy as np

for _p in ("/opt/trn_rl_repo",):
    if os.path.isdir(_p) and _p not in sys.path:
        sys.path.append(_p)

H = W = 512
G = 256
NCORES = 8
SH = H // NCORES      # strip rows per core = 64
TR, TC = 16, 32       # tile rows x cols
N = TR * TC           # 512 pixels per tile
TPR = W // TC         # tiles per strip row = 16
TRW = SH // TR        # tile rows per strip = 4
NT = TRW * TPR        # tiles per core = 64
KS = TR + TC          # one-hot rows for the S matmul
TS = 5                # max tiles (slots) per pack
GRP = 4               # packs per psum output group (4*15=60 rows)
PI = float(np.pi)
CULL_THR = 5e-3
WCAP = 5e4            # fp16 row-scale cap for E weights

_PROGRAMS = {}


def _build_program(npack, groups):
    from concourse import bacc, mybir, tile

    f32 = mybir.dt.float32
    f16 = mybir.dt.float16
    Act = mybir.ActivationFunctionType
    Alu = mybir.AluOpType

    nc = bacc.Bacc("TRN2", target_bir_lowering=False, debug=False,
                   num_devices=NCORES)

    # feat/onehot ride in the first 4 pack-slots of the weight tensors so a
    # single DMA unblocks the first matmul.
    wed = nc.dram_tensor("we", [12, npack + 4, 128], f16, kind="ExternalInput")
    escd = nc.dram_tensor("esc", [128, npack // 2], f32, kind="ExternalInput")
    wsd = nc.dram_tensor("ws", [KS, npack + 4, 128], f16,
                         kind="ExternalInput")
    abd = nc.dram_tensor("ab", [128, npack, 2, 3 * TS * GRP], f16,
                         kind="ExternalInput")
    ngrp = len(groups)
    outd = nc.dram_tensor("out", [ngrp, 3 * TS * GRP, N], f32,
                          kind="ExternalOutput")

    with tile.TileContext(nc) as tc:
        with (
            tc.tile_pool(name="io", bufs=1) as iop,
            tc.tile_pool(name="trig", bufs=3) as trigp,
            tc.tile_pool(name="prod", bufs=3) as pp,
            tc.tile_pool(name="m2", bufs=3, space="PSUM") as m2p,
            tc.tile_pool(name="po", bufs=2, space="PSUM") as pop,
        ):
            # spread input loads over three DGE queues so issue overlaps
            we_sb = iop.tile([12, npack + 4, 128], f16, tag="we")
            nc.sync.dma_start(out=we_sb[:], in_=wed[:])
            esc_sb = iop.tile([128, npack // 2], f32, tag="esc")
            nc.sync.dma_start(out=esc_sb[:], in_=escd[:])
            ws_sb = iop.tile([KS, npack + 4, 128], f16, tag="ws")
            nc.scalar.dma_start(out=ws_sb[:], in_=wsd[:])
            ab_sb = iop.tile([128, npack, 2, 3 * TS * GRP], f16, tag="ab")
            nc.gpsimd.dma_start(out=ab_sb[:], in_=abd[:])
            feat_sb = we_sb[:, 0:4]          # [12, 4, 128] == [12, N]
            oh_sb = ws_sb[:, 0:4]            # [KS, 4, 128] == [KS, N]
            gauss = iop.tile([128, npack, N], f16, tag="gauss")
            wall = iop.tile([128, npack, N], f16, tag="wall")

            # Phase A per superpack: E+S matmuls, Exp (Exp table), wrap.
            def emit_a(sp):
                mE = m2p.tile([128, 2, N], f32, tag="m2", name="mE")
                for h in range(2):
                    pk = 2 * sp + h
                    nc.tensor.matmul(mE[:, h], we_sb[:, 4 + pk], feat_sb,
                                     start=True, stop=True)
                nc.scalar.activation(gauss[:, 2 * sp:2 * sp + 2], mE[:],
                                     Act.Exp, scale=esc_sb[:, sp:sp + 1])
                mS = m2p.tile([128, 2, N], f32, tag="m2", name="mS")
                for h in range(2):
                    nc.tensor.matmul(mS[:, h], ws_sb[:, 4 + 2 * sp + h],
                                     oh_sb, start=True, stop=True)
                nc.vector.add_range_wrap(wall[:, 2 * sp:2 * sp + 2], mS[:],
                                         0.0, PI, 2.0 * PI)

            # Phase B per output group (<=4 packs, Sin table): sinusoids,
            # products, reduction, flush.
            def emit_b(g0, gs):
                po = pop.tile([3 * TS * gs, N], f32, tag="po", name="po")
                wsl = wall[:, g0:g0 + gs]
                gsl = gauss[:, g0:g0 + gs]
                t2 = trigp.tile([128, gs, N], f16, tag="t2", name="t2")
                nc.scalar.activation(t2[:], wsl, Act.Sin, scale=0.5)
                ss = trigp.tile([128, gs, N], f16, tag="ss", name="ss")
                nc.scalar.activation(ss[:], wsl, Act.Sin)
                p2 = pp.tile([128, gs, N], f16, tag="p2", name="p2")
                nc.vector.tensor_mul(p2[:], gsl, ss[:])
                uu = trigp.tile([128, gs, N], f16, tag="u", name="u")
                nc.vector.tensor_mul(uu[:], t2[:], t2[:])
                p1 = pp.tile([128, gs, N], f16, tag="p1", name="p1")
                nc.vector.scalar_tensor_tensor(
                    p1[:], uu[:], 0.5, gsl, Alu.subtract, Alu.mult)
                for i in range(gs):
                    pk = g0 + i
                    nc.tensor.matmul(
                        po[:], ab_sb[:, pk, 1, :3 * TS * gs], p2[:, i],
                        start=(i == 0), stop=False)
                for i in range(gs):
                    pk = g0 + i
                    nc.tensor.matmul(
                        po[:], ab_sb[:, pk, 0, :3 * TS * gs], p1[:, i],
                        start=False, stop=(i == gs - 1))
                ob = pp.tile([3 * TS * gs, N], f32, tag="ob", name="ob")
                nc.vector.tensor_scalar_add(ob[:], po[:], 0.0)
                grp = g0 // GRP
                nc.sync.dma_start(out=outd[grp, :3 * TS * gs, :], in_=ob[:])

            # Interleave in two half-blocks: the PE/DVE queues overlap
            # phase-B groups with the second half's phase-A work while the
            # ACT queue only pays ~2 extra table switches.
            nsp = npack // 2
            half_sps = nsp      # single block: scheduler interleaves on its own
            done = 0
            for target in (half_sps, nsp):
                for sp in range(done, target):
                    emit_a(sp)
                for g0, gs in groups:
                    last_sp = (g0 + gs - 1) // 2
                    if done <= last_sp < target or (
                            target == nsp and last_sp >= nsp):
                        emit_b(g0, gs)
                done = target

    nc.compile()
    return nc


def _wrap(x):
    return np.mod(x + np.pi, 2.0 * np.pi) - np.pi


def _pack_tiles(mk):
    """First-fit-decreasing bin packing of tiles into 128-row packs with at
    most TS tiles each. mk[t] = row budget of tile t (shared across cores).
    Returns list of packs, each a list of (tile, row_offset)."""
    order = np.argsort(-mk, kind="stable")
    packs = []      # [rows_used, [(t, off)]]
    for t in order:
        need = int(mk[t])
        placed = False
        for p in packs:
            if p[0] + need <= 128 and len(p[1]) < TS:
                p[1].append((int(t), p[0]))
                p[0] += need
                placed = True
                break
        if not placed:
            packs.append([need, [(int(t), 0)]])
    return [p[1] for p in packs]


def _host_arrays(inputs):
    gx = np.asarray(inputs["grid_x"], np.float64)
    gy = np.asarray(inputs["grid_y"], np.float64)
    u = np.clip(np.asarray(inputs["u"], np.float64), -1, 1)
    v = np.clip(np.asarray(inputs["v"], np.float64), -1, 1)
    th = np.clip(np.asarray(inputs["theta"], np.float64), -2, 2) * (2 * np.pi)
    sig = np.clip(np.asarray(inputs["rel_sigma"], np.float64), 0.001, 1.0)
    rf = np.clip(np.asarray(inputs["rel_freq"], np.float64), -5, 5)
    gam = np.clip(np.asarray(inputs["gamma"], np.float64), 0.0001, 1.0)
    psi = np.clip(np.asarray(inputs["psi"], np.float64), -1, 1)
    amp = np.clip(np.asarray(inputs["amplitude"], np.float64), 0, 1)

    cr, sr = np.cos(th), np.sin(th)
    cx = -(cr * u + sr * v)       # x_rot = cr*X + sr*Y + cx
    cy = sr * u - cr * v
    p = 1.0 / (2.0 * sig * sig)
    q = 1.0 / (2.0 * gam * gam)
    freq = 2 * np.pi / np.exp(rf)
    phase = psi * (2 * np.pi)
    alpha = amp * np.cos(phase)                   # [G,3]
    beta = -amp * np.sin(phase)

    ampmax = amp.max(1)
    elim = np.log(np.maximum(CULL_THR / np.maximum(ampmax, 1e-30), 1e-300))

    # --- per (core, tile) gabor culling: exact per-pixel E max over tile.
    crf = cr.astype(np.float32)[:, None, None]
    srf = sr.astype(np.float32)[:, None, None]
    pf = p.astype(np.float32)[:, None, None]
    qf = q.astype(np.float32)[:, None, None]
    keeps = []                     # keeps[core][t] = gabor index array
    for core in range(NCORES):
        Xs = np.asarray(gx[core * SH:(core + 1) * SH], np.float32)
        Ys = np.asarray(gy[core * SH:(core + 1) * SH], np.float32)
        dx = Xs[None] - u.astype(np.float32)[:, None, None]
        dy = Ys[None] - v.astype(np.float32)[:, None, None]
        xr = dx * crf + dy * srf
        yr = dy * crf - dx * srf
        E = -(xr * xr * pf + yr * yr * qf)
        Em = E.reshape(G, TRW, TR, TPR, TC).max(axis=(2, 4))   # [G,4,16]
        keeps.append([np.flatnonzero(Em[:, t // TPR, t % TPR] >= elim)
                      for t in range(NT)])

    kmat = np.array([[len(keeps[c][t]) for t in range(NT)]
                     for c in range(NCORES)])
    mk = np.maximum(kmat.max(axis=0), 1)           # shared row budget
    packs = _pack_tiles(mk)
    npack = len(packs)
    if npack % 2:
        packs.append([])
        npack += 1
    groups = []                                     # (first_pack, size)
    g0 = 0
    while g0 < npack:
        groups.append((g0, min(GRP, npack - g0)))
        g0 += GRP
    assert all(gs % 2 == 0 for _, gs in groups)

    # fp16-exact feature basis: f1=(2c-31)/1024, f2=(2r-15)/1024;
    # dx = K*f1, dy = K*f2 with K = 512/511 folded into the weights.
    # Rows duplicated so one K=12 matmul accumulates the hi+lo weights.
    c_i = np.tile(np.arange(TC), TR)
    r_i = np.repeat(np.arange(TR), TC)
    f1 = (2 * c_i - 31) / 1024.0
    f2 = (2 * r_i - 15) / 1024.0
    feat6 = np.stack([f1, f2, np.ones_like(f1), f1 * f1, f2 * f2, f1 * f2],
                     0).astype(np.float16)
    feat = np.concatenate([feat6, feat6], 0)
    K1 = 512.0 / 511.0
    xs = gx[0]
    ys = gy[:, 0]
    onehot = np.zeros((KS, N), np.float16)
    onehot[r_i, np.arange(N)] = 1.0
    onehot[TR + c_i, np.arange(N)] = 1.0
    Xc_col = xs.reshape(TPR, TC).mean(1)                   # per tile-col
    Yc_row = ys.reshape(H // TR, TR).mean(1)               # per global tile-row
    yoff = ys[:TR] - ys[:TR].mean()                        # [TR]
    xoff = xs[:TC] - xs[:TC].mean()                        # [TC]

    # map (pack, slot) -> tile and po-row base; shared across cores
    tile_map = []                  # (grp, row_base, tile)
    for pi, pk in enumerate(packs):
        grp = pi // GRP
        ib = (pi % GRP) * 3 * TS
        for s, (t, off) in enumerate(pk):
            tile_map.append((grp, ib + 3 * s, t))

    in_maps = []
    for core in range(NCORES):
        WE = np.zeros((6, npack, 128), np.float64)
        WS = np.zeros((KS, npack, 128), np.float16)
        AB = np.zeros((128, npack, 2, 3 * TS * GRP), np.float16)
        for pi, pk in enumerate(packs):
            ib = (pi % GRP) * 3 * TS
            for s, (t, off) in enumerate(pk):
                g_ids = keeps[core][t]
                k = len(g_ids)
                if k == 0:
                    continue
                trow, tcol = divmod(t, TPR)
                Xc = Xc_col[tcol]
                Yc = Yc_row[core * TRW + trow]
                crk, srk = cr[g_ids], sr[g_ids]
                pk_, qk = p[g_ids], q[g_ids]
                cxt = Xc * crk + Yc * srk + cx[g_ids]
                cyt = -Xc * srk + Yc * crk + cy[g_ids]
                rows = slice(off, off + k)
                WE[0, pi, rows] = -(2 * pk_ * crk * cxt
                                    - 2 * qk * srk * cyt) * K1
                WE[1, pi, rows] = -(2 * pk_ * srk * cxt
                                    + 2 * qk * crk * cyt) * K1
                WE[2, pi, rows] = -(pk_ * cxt * cxt + qk * cyt * cyt)
                WE[3, pi, rows] = -(pk_ * crk * crk + qk * srk * srk) * K1 * K1
                WE[4, pi, rows] = -(pk_ * srk * srk + qk * crk * crk) * K1 * K1
                WE[5, pi, rows] = -2 * crk * srk * (pk_ - qk) * K1 * K1
                fk = freq[g_ids]
                A = _wrap(fk[:, None] * srk[:, None] * yoff[None, :])
                Bt = _wrap(fk[:, None] * crk[:, None] * xoff[None, :]
                           + (fk * cxt)[:, None])
                WS[:TR, pi, rows] = A.T
                WS[TR:, pi, rows] = Bt.T
                for ch in range(3):
                    AB[rows, pi, 0, ib + 3 * s + ch] = -2 * alpha[g_ids, ch]
                    AB[rows, pi, 1, ib + 3 * s + ch] = beta[g_ids, ch]

        # per-row power-of-2 scale (shared within a superpack) into fp16
        m = np.abs(WE).max(axis=0)                         # [npack, 128]
        m2 = m.reshape(npack // 2, 2, 128).max(axis=1)     # [nsp, 128]
        lam = np.exp2(-np.maximum(0.0, np.ceil(np.log2(
            np.maximum(m2, 1e-30) / WCAP))))               # [nsp, 128]
        lam_pk = np.repeat(lam, 2, axis=0)                 # [npack, 128]
        WEs = WE * lam_pk[None, :, :]
        WEh = WEs.astype(np.float16)
        WEl = (WEs - WEh.astype(np.float64)).astype(np.float16)
        ESC = np.ascontiguousarray((1.0 / lam).T.astype(np.float32))

        WE12 = np.concatenate([WEh, WEl], 0)               # [12, npack, 128]
        we_full = np.concatenate(
            [feat.reshape(12, 4, 128), WE12], 1)           # [12, npack+4, 128]
        ws_full = np.concatenate(
            [onehot.reshape(KS, 4, 128), WS], 1)           # [KS, npack+4, 128]
        in_maps.append({
            "we": np.ascontiguousarray(we_full),
            "esc": ESC,
            "ws": np.ascontiguousarray(ws_full),
            "ab": np.ascontiguousarray(AB),
        })
    return in_maps, npack, tuple(groups), tile_map


def _get_program(npack, groups):
    key = (npack, groups)
    if key not in _PROGRAMS:
        _PROGRAMS[key] = _build_program(npack, groups)
    return _PROGRAMS[key]


def kernel(**inputs):
    from concourse.bass_utils import run_bass_kernel_spmd

    in_maps, npack, groups, tile_map = _host_arrays(inputs)
    nc = _get_program(npack, groups)
    res = run_bass_kernel_spmd(nc, in_maps, list(range(NCORES)))
    out = np.empty((3, H, W), np.float32)
    for core in range(NCORES):
        o = res.results[core]["out"]               # [ngrp, 60, N]
        for grp, rb, t in tile_map:
            trow, tcol = divmod(t, TPR)
            out[:, core * SH + trow * TR:core * SH + (trow + 1) * TR,
                tcol * TC:(tcol + 1) * TC] = \
                o[grp, rb:rb + 3].reshape(3, TR, TC)
    np.clip(out, -1.0, 1.0, out=out)
    return out


# revision 28
# speedup vs baseline: 1.0097x; 1.0097x over previous
"""Gabor layer Trainium2 kernel — packed-tile formulation.

Per gabor g and pixel (x,y): amp[g,c] * exp(E) * cos(S + phase[g,c]) with E
quadratic and S affine in pixel coords. Using cos(S+p) = cos(p)cos(S) -
sin(p)sin(S), the channel sum over g is a matmul over the plane pair
(cos(S)*gauss, sin(S)*gauss) with contraction over gabors.

Key observation: with tile-centered features on a uniform grid, the matmul
rhs (tile-local monomials for E; row/col one-hots for S) is IDENTICAL for
every 16x32 tile — all per-tile variation lives in the stationary weight
tables. So rows of one 128-partition work unit can belong to DIFFERENT
tiles: each partition row is a (gabor, tile) pair. Gabors are culled per
tile (exact per-pixel E max, thr=1e-3), tiles are bin-packed ~4 per
128-row "pack", and every engine pass (matmul / Exp / Sin / products /
reduce) is amortized over all tiles in the pack.

E matmul runs in fp16 hi/lo (2 accumulated passes) instead of fp32 (2
half-speed passes): the feature basis f=(2c-31)/1024 etc. is chosen
fp16-EXACT, weights are scaled per row by a power of two into fp16 range
(quadratic coefficients reach 5e7 for thin gabors), and the inverse scale
rides the Exp activation's per-partition scale operand.

Device pipeline per pack (N=512 px):
    PE : E = (WEhi+WElo)^T @ feat6   (K=6 fp16 x2, row-scaled)
         S = WS^T @ onehot           (K=48 fp16: S[px] = A[g,row]+B[g,col])
    ACT: gauss = Exp(E * invscale)   (fp16)      [Exp table]
    DVE: w = add_range_wrap(S) in [-pi,pi)  (fp16; w == S mod 2pi)
    ACT: t2 = Sin(w*0.5)  (sin^2(w/2) == sin^2(S/2));  ss = Sin(w)
    DVE: u = t2*t2;  p1 = (u-0.5)*gauss = -cos(S)/2*gauss;  p2 = gauss*ss
    PE : po[60,512] += AB^T @ [p1;p2]  (fp16 zero-col-padded AB columns
         accumulate a 4-pack group into ONE psum bank)
    DVE: ob = copy(po);  1 DMA out per 4-pack group.
Final clamp + tile unscramble on host.

Sharding: each core owns a 64-row strip; the tile->pack map is shared
across cores (per-tile row budget = max gabor count over cores) so the
single SPMD program's baked addresses are valid on every core.
"""

import os
import sys

import numpy as np

for _p in ("/opt/trn_rl_repo",):
    if os.path.isdir(_p) and _p not in sys.path:
        sys.path.append(_p)

H = W = 512
G = 256
NCORES = 8
SH = H // NCORES      # strip rows per core = 64
TR, TC = 16, 32       # tile rows x cols
N = TR * TC           # 512 pixels per tile
TPR = W // TC         # tiles per strip row = 16
TRW = SH // TR        # tile rows per strip = 4
NT = TRW * TPR        # tiles per core = 64
KS = TR + TC          # one-hot rows for the S matmul
TS = 5                # max tiles (slots) per pack
GRP = 8               # packs per psum output group (8*15=120 rows)
BAT = 4               # packs per phase-B compute batch
PI = float(np.pi)
CULL_THR = 3e-3
WCAP = 5e4            # fp16 row-scale cap for E weights

_PROGRAMS = {}


def _build_program(npack, groups):
    from concourse import bacc, mybir, tile

    f32 = mybir.dt.float32
    f16 = mybir.dt.float16
    Act = mybir.ActivationFunctionType
    Alu = mybir.AluOpType

    nc = bacc.Bacc("TRN2", target_bir_lowering=False, debug=False,
                   num_devices=NCORES)

    # feat/onehot ride in the first 4 pack-slots of the weight tensors so a
    # single DMA unblocks the first matmul.
    wed = nc.dram_tensor("we", [12, npack + 4, 128], f16, kind="ExternalInput")
    escd = nc.dram_tensor("esc", [128, npack // 2], f32, kind="ExternalInput")
    wsd = nc.dram_tensor("ws", [KS, npack + 4, 128], f16,
                         kind="ExternalInput")
    abd = nc.dram_tensor("ab", [128, npack, 2, 3 * TS * GRP], f16,
                         kind="ExternalInput")
    ngrp = len(groups)
    outd = nc.dram_tensor("out", [ngrp, 3 * TS * GRP, N], f32,
                          kind="ExternalOutput")

    with tile.TileContext(nc) as tc:
        with (
            tc.tile_pool(name="io", bufs=1) as iop,
            tc.tile_pool(name="trig", bufs=3) as trigp,
            tc.tile_pool(name="prod", bufs=3) as pp,
            tc.tile_pool(name="m2", bufs=3, space="PSUM") as m2p,
            tc.tile_pool(name="po", bufs=2, space="PSUM") as pop,
        ):
            # spread input loads over three DGE queues so issue overlaps
            we_sb = iop.tile([12, npack + 4, 128], f16, tag="we")
            nc.sync.dma_start(out=we_sb[:], in_=wed[:])
            esc_sb = iop.tile([128, npack // 2], f32, tag="esc")
            nc.sync.dma_start(out=esc_sb[:], in_=escd[:])
            ws_sb = iop.tile([KS, npack + 4, 128], f16, tag="ws")
            nc.scalar.dma_start(out=ws_sb[:], in_=wsd[:])
            ab_sb = iop.tile([128, npack, 2, 3 * TS * GRP], f16, tag="ab")
            nc.gpsimd.dma_start(out=ab_sb[:], in_=abd[:])
            feat_sb = we_sb[:, 0:4]          # [12, 4, 128] == [12, N]
            oh_sb = ws_sb[:, 0:4]            # [KS, 4, 128] == [KS, N]
            gauss = iop.tile([128, npack, N], f16, tag="gauss")
            wall = iop.tile([128, npack, N], f16, tag="wall")

            # Phase A per superpack: E+S matmuls, Exp (Exp table), wrap.
            def emit_a(sp):
                mE = m2p.tile([128, 2, N], f32, tag="m2", name="mE")
                for h in range(2):
                    pk = 2 * sp + h
                    nc.tensor.matmul(mE[:, h], we_sb[:, 4 + pk], feat_sb,
                                     start=True, stop=True)
                nc.scalar.activation(gauss[:, 2 * sp:2 * sp + 2], mE[:],
                                     Act.Exp, scale=esc_sb[:, sp:sp + 1])
                mS = m2p.tile([128, 2, N], f32, tag="m2", name="mS")
                for h in range(2):
                    nc.tensor.matmul(mS[:, h], ws_sb[:, 4 + 2 * sp + h],
                                     oh_sb, start=True, stop=True)
                nc.vector.add_range_wrap(wall[:, 2 * sp:2 * sp + 2], mS[:],
                                         0.0, PI, 2.0 * PI)

            # Phase B per output group (<=8 packs, Sin table): sinusoids and
            # products in 4-pack batches, reduction into one psum bank,
            # single flush per group.
            def emit_b(g0, gs):
                po = pop.tile([3 * TS * gs, N], f32, tag="po", name="po")
                for b0 in range(0, gs, BAT):
                    bs = min(BAT, gs - b0)
                    wsl = wall[:, g0 + b0:g0 + b0 + bs]
                    gsl = gauss[:, g0 + b0:g0 + b0 + bs]
                    t2 = trigp.tile([128, bs, N], f16, tag="t2", name="t2")
                    nc.scalar.activation(t2[:], wsl, Act.Sin, scale=0.5)
                    ss = trigp.tile([128, bs, N], f16, tag="ss", name="ss")
                    nc.scalar.activation(ss[:], wsl, Act.Sin)
                    p2 = pp.tile([128, bs, N], f16, tag="p2", name="p2")
                    nc.vector.tensor_mul(p2[:], gsl, ss[:])
                    uu = trigp.tile([128, bs, N], f16, tag="u", name="u")
                    nc.vector.tensor_mul(uu[:], t2[:], t2[:])
                    p1 = pp.tile([128, bs, N], f16, tag="p1", name="p1")
                    nc.vector.scalar_tensor_tensor(
                        p1[:], uu[:], 0.5, gsl, Alu.subtract, Alu.mult)
                    for i in range(bs):
                        nc.tensor.matmul(
                            po[:], ab_sb[:, g0 + b0 + i, 1, :3 * TS * gs],
                            p2[:, i], start=(b0 + i == 0), stop=False)
                    for i in range(bs):
                        nc.tensor.matmul(
                            po[:], ab_sb[:, g0 + b0 + i, 0, :3 * TS * gs],
                            p1[:, i], start=False,
                            stop=(b0 + bs == gs and i == bs - 1))
                ob = pp.tile([3 * TS * gs, N], f32, tag="ob", name="ob")
                nc.vector.tensor_scalar_add(ob[:], po[:], 0.0)
                grp = g0 // GRP
                nc.sync.dma_start(out=outd[grp, :3 * TS * gs, :], in_=ob[:])

            # Interleave in two half-blocks: the PE/DVE queues overlap
            # phase-B groups with the second half's phase-A work while the
            # ACT queue only pays ~2 extra table switches.
            nsp = npack // 2
            half_sps = nsp      # single block: scheduler interleaves on its own
            done = 0
            for target in (half_sps, nsp):
                for sp in range(done, target):
                    emit_a(sp)
                for g0, gs in groups:
                    last_sp = (g0 + gs - 1) // 2
                    if done <= last_sp < target or (
                            target == nsp and last_sp >= nsp):
                        emit_b(g0, gs)
                done = target

    nc.compile()
    return nc


def _wrap(x):
    return np.mod(x + np.pi, 2.0 * np.pi) - np.pi


def _pack_tiles(mk):
    """First-fit-decreasing bin packing of tiles into 128-row packs with at
    most TS tiles each. mk[t] = row budget of tile t (shared across cores).
    Returns list of packs, each a list of (tile, row_offset)."""
    order = np.argsort(-mk, kind="stable")
    packs = []      # [rows_used, [(t, off)]]
    for t in order:
        need = int(mk[t])
        placed = False
        for p in packs:
            if p[0] + need <= 128 and len(p[1]) < TS:
                p[1].append((int(t), p[0]))
                p[0] += need
                placed = True
                break
        if not placed:
            packs.append([need, [(int(t), 0)]])
    return [p[1] for p in packs]


def _host_arrays(inputs):
    gx = np.asarray(inputs["grid_x"], np.float64)
    gy = np.asarray(inputs["grid_y"], np.float64)
    u = np.clip(np.asarray(inputs["u"], np.float64), -1, 1)
    v = np.clip(np.asarray(inputs["v"], np.float64), -1, 1)
    th = np.clip(np.asarray(inputs["theta"], np.float64), -2, 2) * (2 * np.pi)
    sig = np.clip(np.asarray(inputs["rel_sigma"], np.float64), 0.001, 1.0)
    rf = np.clip(np.asarray(inputs["rel_freq"], np.float64), -5, 5)
    gam = np.clip(np.asarray(inputs["gamma"], np.float64), 0.0001, 1.0)
    psi = np.clip(np.asarray(inputs["psi"], np.float64), -1, 1)
    amp = np.clip(np.asarray(inputs["amplitude"], np.float64), 0, 1)

    cr, sr = np.cos(th), np.sin(th)
    cx = -(cr * u + sr * v)       # x_rot = cr*X + sr*Y + cx
    cy = sr * u - cr * v
    p = 1.0 / (2.0 * sig * sig)
    q = 1.0 / (2.0 * gam * gam)
    freq = 2 * np.pi / np.exp(rf)
    phase = psi * (2 * np.pi)
    alpha = amp * np.cos(phase)                   # [G,3]
    beta = -amp * np.sin(phase)

    ampmax = amp.max(1)
    elim = np.log(np.maximum(CULL_THR / np.maximum(ampmax, 1e-30), 1e-300))

    # --- per (core, tile) gabor culling: exact per-pixel E max over tile.
    crf = cr.astype(np.float32)[:, None, None]
    srf = sr.astype(np.float32)[:, None, None]
    pf = p.astype(np.float32)[:, None, None]
    qf = q.astype(np.float32)[:, None, None]
    keeps = []                     # keeps[core][t] = gabor index array
    for core in range(NCORES):
        Xs = np.asarray(gx[core * SH:(core + 1) * SH], np.float32)
        Ys = np.asarray(gy[core * SH:(core + 1) * SH], np.float32)
        dx = Xs[None] - u.astype(np.float32)[:, None, None]
        dy = Ys[None] - v.astype(np.float32)[:, None, None]
        xr = dx * crf + dy * srf
        yr = dy * crf - dx * srf
        E = -(xr * xr * pf + yr * yr * qf)
        Em = E.reshape(G, TRW, TR, TPR, TC).max(axis=(2, 4))   # [G,4,16]
        keeps.append([np.flatnonzero(Em[:, t // TPR, t % TPR] >= elim)
                      for t in range(NT)])

    kmat = np.array([[len(keeps[c][t]) for t in range(NT)]
                     for c in range(NCORES)])
    mk = np.maximum(kmat.max(axis=0), 1)           # shared row budget
    packs = _pack_tiles(mk)
    npack = len(packs)
    if npack % 2:
        packs.append([])
        npack += 1
    groups = []                                     # (first_pack, size)
    g0 = 0
    while g0 < npack:
        groups.append((g0, min(GRP, npack - g0)))
        g0 += GRP
    assert all(gs % 2 == 0 for _, gs in groups)

    # fp16-exact feature basis: f1=(2c-31)/1024, f2=(2r-15)/1024;
    # dx = K*f1, dy = K*f2 with K = 512/511 folded into the weights.
    # Rows duplicated so one K=12 matmul accumulates the hi+lo weights.
    c_i = np.tile(np.arange(TC), TR)
    r_i = np.repeat(np.arange(TR), TC)
    f1 = (2 * c_i - 31) / 1024.0
    f2 = (2 * r_i - 15) / 1024.0
    feat6 = np.stack([f1, f2, np.ones_like(f1), f1 * f1, f2 * f2, f1 * f2],
                     0).astype(np.float16)
    feat = np.concatenate([feat6, feat6], 0)
    K1 = 512.0 / 511.0
    xs = gx[0]
    ys = gy[:, 0]
    onehot = np.zeros((KS, N), np.float16)
    onehot[r_i, np.arange(N)] = 1.0
    onehot[TR + c_i, np.arange(N)] = 1.0
    Xc_col = xs.reshape(TPR, TC).mean(1)                   # per tile-col
    Yc_row = ys.reshape(H // TR, TR).mean(1)               # per global tile-row
    yoff = ys[:TR] - ys[:TR].mean()                        # [TR]
    xoff = xs[:TC] - xs[:TC].mean()                        # [TC]

    # map (pack, slot) -> tile and po-row base; shared across cores
    tile_map = []                  # (grp, row_base, tile)
    for pi, pk in enumerate(packs):
        grp = pi // GRP
        ib = (pi % GRP) * 3 * TS
        for s, (t, off) in enumerate(pk):
            tile_map.append((grp, ib + 3 * s, t))

    in_maps = []
    for core in range(NCORES):
        WE = np.zeros((6, npack, 128), np.float64)
        WS = np.zeros((KS, npack, 128), np.float16)
        AB = np.zeros((128, npack, 2, 3 * TS * GRP), np.float16)
        for pi, pk in enumerate(packs):
            ib = (pi % GRP) * 3 * TS
            for s, (t, off) in enumerate(pk):
                g_ids = keeps[core][t]
                k = len(g_ids)
                if k == 0:
                    continue
                trow, tcol = divmod(t, TPR)
                Xc = Xc_col[tcol]
                Yc = Yc_row[core * TRW + trow]
                crk, srk = cr[g_ids], sr[g_ids]
                pk_, qk = p[g_ids], q[g_ids]
                cxt = Xc * crk + Yc * srk + cx[g_ids]
                cyt = -Xc * srk + Yc * crk + cy[g_ids]
                rows = slice(off, off + k)
                WE[0, pi, rows] = -(2 * pk_ * crk * cxt
                                    - 2 * qk * srk * cyt) * K1
                WE[1, pi, rows] = -(2 * pk_ * srk * cxt
                                    + 2 * qk * crk * cyt) * K1
                WE[2, pi, rows] = -(pk_ * cxt * cxt + qk * cyt * cyt)
                WE[3, pi, rows] = -(pk_ * crk * crk + qk * srk * srk) * K1 * K1
                WE[4, pi, rows] = -(pk_ * srk * srk + qk * crk * crk) * K1 * K1
                WE[5, pi, rows] = -2 * crk * srk * (pk_ - qk) * K1 * K1
                fk = freq[g_ids]
                A = _wrap(fk[:, None] * srk[:, None] * yoff[None, :])
                Bt = _wrap(fk[:, None] * crk[:, None] * xoff[None, :]
                           + (fk * cxt)[:, None])
                WS[:TR, pi, rows] = A.T
                WS[TR:, pi, rows] = Bt.T
                for ch in range(3):
                    AB[rows, pi, 0, ib + 3 * s + ch] = -2 * alpha[g_ids, ch]
                    AB[rows, pi, 1, ib + 3 * s + ch] = beta[g_ids, ch]

        # per-row power-of-2 scale (shared within a superpack) into fp16
        m = np.abs(WE).max(axis=0)                         # [npack, 128]
        m2 = m.reshape(npack // 2, 2, 128).max(axis=1)     # [nsp, 128]
        lam = np.exp2(-np.maximum(0.0, np.ceil(np.log2(
            np.maximum(m2, 1e-30) / WCAP))))               # [nsp, 128]
        lam_pk = np.repeat(lam, 2, axis=0)                 # [npack, 128]
        WEs = WE * lam_pk[None, :, :]
        WEh = WEs.astype(np.float16)
        WEl = (WEs - WEh.astype(np.float64)).astype(np.float16)
        ESC = np.ascontiguousarray((1.0 / lam).T.astype(np.float32))

        WE12 = np.concatenate([WEh, WEl], 0)               # [12, npack, 128]
        we_full = np.concatenate(
            [feat.reshape(12, 4, 128), WE12], 1)           # [12, npack+4, 128]
        ws_full = np.concatenate(
            [onehot.reshape(KS, 4, 128), WS], 1)           # [KS, npack+4, 128]
        in_maps.append({
            "we": np.ascontiguousarray(we_full),
            "esc": ESC,
            "ws": np.ascontiguousarray(ws_full),
            "ab": np.ascontiguousarray(AB),
        })
    return in_maps, npack, tuple(groups), tile_map


def _get_program(npack, groups):
    key = (npack, groups)
    if key not in _PROGRAMS:
        _PROGRAMS[key] = _build_program(npack, groups)
    return _PROGRAMS[key]


def kernel(**inputs):
    from concourse.bass_utils import run_bass_kernel_spmd

    in_maps, npack, groups, tile_map = _host_arrays(inputs)
    nc = _get_program(npack, groups)
    res = run_bass_kernel_spmd(nc, in_maps, list(range(NCORES)))
    out = np.empty((3, H, W), np.float32)
    for core in range(NCORES):
        o = res.results[core]["out"]               # [ngrp, 60, N]
        for grp, rb, t in tile_map:
            trow, tcol = divmod(t, TPR)
            out[:, core * SH + trow * TR:core * SH + (trow + 1) * TR,
                tcol * TC:(tcol + 1) * TC] = \
                o[grp, rb:rb + 3].reshape(3, TR, TC)
    np.clip(out, -1.0, 1.0, out=out)
    return out
